# revision 4
# baseline (speedup 1.0000x reference)
"""Corr1d (stereo cost volume) Trainium2 kernel, v4.

corrmap[b, i, h, w] = sum_c fL[b, c, h, w] * fR[b, c, h, w - i],  i in [0, 64)
Shapes: fL, fR [8, 128, 160, 320] f32 -> corrmap [8, 64, 160, 320] f32.
Sharding: data-parallel over batch; core k handles batch element k.
Host: f32->bf16 (RTNE) before upload; bf16 on device; upcast f32 on host.

v4 vs v2 (256us baseline):
  * One dump per batch: scratch rows hold (hc2, g, n, h8) interleaved, so
    the whole band (all 3 tile-groups) dumps in one DMA with 9KB runs.
    g2's matmuls sit at PSUM partitions 64-127 (tile_position=(0,64/96))
    so its band lives on b01 partitions 64-127.
  * Per-tile -256 element shift in the dump AP makes the diagonal readback
    address affine in the partition index: readback per group is one 3-dim
    DMA [[GP3-8,128],[3*ROW,2],[1,512]] covering both hc2 chunks.
  * Dump+readbacks alternate sync/scalar HWDGE rings per batch (same-ring
    FIFO preserves the DRAM dump->readback ordering; two batches in flight
    use different rings).
  * Both input loads on gpsimd (deep queue), stores spread over all rings,
    3-deep software pipeline.

Self-contained: shapes hardcoded; requires only numpy + ml_dtypes + concourse.
"""

import ml_dtypes
import numpy as np

import concourse.bacc as bacc
import concourse.bass as bass
import concourse.mybir as mybir
from concourse.bass_utils import run_bass_kernel_spmd
from concourse.tile import TileContext
from concourse.masks import make_identity

F32 = mybir.dt.float32
BF16 = mybir.dt.bfloat16

N_CORES = 8
C = 128           # channels (matmul contraction dim)
H = 160
W = 320
D = 64            # disparities
NH = 16           # h rows per batch
NB = H // NH      # batches (10)
NS = 95           # band columns per 32-wide w-tile (32 + 63)
SC = 96           # stored band columns per tile row
FRPAD = 64        # zero pad columns at the start of the fR buffer
HW = H * W
NC2 = NH // 8     # hc2 chunks per batch (2)
ROW = SC * 8      # 768 elements per (p, hc, g)
GROW = 3 * ROW    # 2304: (g, n, h8) per (p, hc)
GP3 = (H // 8) * GROW + ROW   # 46848: scratch row pitch (+768 shift margin)

_cache = {}


def _build():
    nc = bacc.Bacc("TRN2", target_bir_lowering=False, debug=False,
                   num_devices=N_CORES)
    fL = nc.dram_tensor("fL", [C, H, W], BF16, kind="ExternalInput")
    fR = nc.dram_tensor("fR", [C, H, W], BF16, kind="ExternalInput")
    out = nc.dram_tensor("out", [D, H, W], BF16, kind="ExternalOutput")
    scratch = nc.dram_tensor("scratch", [128, GP3], BF16)

    with TileContext(nc) as tc:
        NLB = 3
        fLb = [nc.alloc_sbuf_tensor(f"fLb{i}", [C, NH * W], BF16)
               for i in range(NLB)]
        fRb = [nc.alloc_sbuf_tensor(f"fRb{i}", [C, FRPAD + NH * W], BF16)
               for i in range(NLB)]
        ident = nc.alloc_sbuf_tensor("ident", [128, 128], BF16)
        make_identity(nc, ident.ap())
        for i in range(NLB):
            nc.vector.memset(fRb[i].ap()[:, 0:FRPAD], 0.0)

        with (
            tc.tile_pool(name="sb", bufs=3) as pool,
            tc.tile_pool(name="ps", bufs=2, space="PSUM") as pp,
        ):
            def emit_loads(b):
                li, ri = fLb[b % NLB], fRb[b % NLB]
                h0 = b * NH
                nc.gpsimd.dma_start(
                    out=li.ap(),
                    in_=bass.AP(fL, h0 * W, [[HW, C], [1, NH * W]]),
                )
                nc.gpsimd.dma_start(
                    out=ri.ap()[:, FRPAD:],
                    in_=bass.AP(fR, h0 * W, [[HW, C], [1, NH * W]]),
                )

            def emit_front(b):
                # matmuls + psum->band copies + garbage memsets + dump +
                # readbacks for batch b; returns the T tiles.
                li, ri = fLb[b % NLB], fRb[b % NLB]
                dr = (nc.sync, nc.scalar)[b % 2]
                # band: [128, (hc2, g, n, h8)]
                bnd = pool.tile([128, NC2 * GROW], BF16, tag="bnd",
                                name=f"bnd_{b}")
                for hq in range(NH // 4):
                    hc2, h8b = hq // 2, 4 * (hq % 2)
                    pss = []
                    for g in range(3):
                        # g2 tiles live at PSUM/band partitions 64-127
                        P0 = 64 if g == 2 else 0
                        nt = 2 if g == 2 else 4
                        ps = pp.tile([128, 4 * NS], F32, tag=f"ps{g}",
                                     name=f"ps{g}_{b}_{hq}")
                        pss.append(ps)
                        for j4 in range(4):
                            hh = 4 * hq + j4
                            for q in range(nt):
                                wt = g * 128 + 32 * q
                                cp = P0 + 32 * q
                                lhsT = bass.AP(li, hh * W + wt,
                                               [[NH * W, C], [1, 32]])
                                rhs = bass.AP(ri, FRPAD + hh * W + wt + 31,
                                              [[FRPAD + NH * W, C], [-1, NS]])
                                nc.tensor.matmul(
                                    ps[cp:cp + 32,
                                       j4 * NS:(j4 + 1) * NS],
                                    lhsT, rhs, start=True, stop=True,
                                    tile_position=(0, cp),
                                )
                    for g in range(3):
                        P0 = 64 if g == 2 else 0
                        P = 64 if g == 2 else 128
                        base = hc2 * GROW + g * ROW + h8b
                        pitch = NC2 * GROW
                        o = bass.AP(bnd.tensor, P0 * pitch + base,
                                    [[pitch, P], [8, NS], [1, 4]])
                        i_ = bass.AP(pss[g].tensor, P0 * (4 * NS),
                                     [[4 * NS, P], [1, NS], [NS, 4]])
                        if (hq + g) % 2 == 0:
                            nc.vector.tensor_copy(out=o, in_=i_)
                        else:
                            nc.scalar.copy(o, i_)
                # zero i > w zones (g0 tiles 0 and 1): band cols n>=32 / n>=64
                pitch = NC2 * GROW
                nc.vector.memset(
                    bass.AP(bnd.tensor, 32 * 8,
                            [[pitch, 32], [GROW, NC2], [1, (SC - 32) * 8]]),
                    0.0)
                nc.vector.memset(
                    bass.AP(bnd.tensor, 32 * pitch + 64 * 8,
                            [[pitch, 32], [GROW, NC2], [1, (SC - 64) * 8]]),
                    0.0)

                # one dump: rows shifted by -256 elements per w-tile index
                dr.dma_start(
                    out=bass.AP(scratch, b * NC2 * GROW,
                                [[32 * GP3 - 256, 4], [GP3, 32],
                                 [1, NC2 * GROW]]),
                    in_=bnd[:, :],
                )
                # diagonal readbacks: T[p, hc2*512 + i*8 + h8]
                # addr = p*(GP3-8) + b*NC2*GROW + g*ROW + hc2*GROW + 248 + ...
                Ts = []
                for g in range(3):
                    P0 = 64 if g == 2 else 0
                    P = 64 if g == 2 else 128
                    T = pool.tile([P, NC2 * 512], BF16, tag=f"T{g}",
                                  name=f"T{g}_{b}")
                    Ts.append(T)
                    dr.dma_start(
                        out=T[:, :],
                        in_=bass.AP(scratch,
                                    P0 * (GP3 - 8) + b * NC2 * GROW
                                    + g * ROW + 31 * 8,
                                    [[GP3 - 8, P], [GROW, NC2], [1, 512]]),
                    )
                return Ts

            def emit_back(b, Ts):
                # transposes + staging copies + output DMAs for batch b
                stg = pool.tile([128, NC2 * 4 * W], BF16, tag="stg",
                                name=f"stg_{b}")
                for hc2 in range(NC2):
                    for a in range(4):
                        u = pp.tile([128, W], BF16, tag="U",
                                    name=f"U_{b}_{hc2}_{a}")
                        cs = hc2 * 512 + 128 * a
                        nc.tensor.transpose(
                            u[:, 0:128], Ts[0][:, cs:cs + 128], ident.ap())
                        nc.tensor.transpose(
                            u[:, 128:256], Ts[1][:, cs:cs + 128], ident.ap())
                        nc.tensor.transpose(
                            u[:, 256:320], Ts[2][:, cs:cs + 128],
                            ident.ap()[0:64, 0:64])
                        o = stg[:, (hc2 * 4 + a) * W:(hc2 * 4 + a + 1) * W]
                        if a % 2 == 0:
                            nc.vector.tensor_copy(out=o, in_=u[:, :])
                        else:
                            nc.scalar.copy(o, u[:, :])
                engs = (nc.sync, nc.scalar, nc.gpsimd,
                        nc.sync, nc.scalar, nc.gpsimd,
                        nc.sync, nc.scalar)
                for hc2 in range(NC2):
                    for a in range(4):
                        k = hc2 * 4 + a
                        engs[k].dma_start(
                            out=bass.AP(out,
                                        16 * a * HW + (NH * b + 8 * hc2) * W,
                                        [[HW, 16], [W, 8], [1, W]]),
                            in_=bass.AP(stg.tensor, k * W,
                                        [[NC2 * 4 * W, 128], [1, W]]),
                        )

            # software pipeline: loads two batches ahead, back one behind
            emit_loads(0)
            emit_loads(1)
            prev = None
            for b in range(NB):
                if b + 2 < NB:
                    emit_loads(b + 2)
                if prev is not None:
                    emit_back(b - 1, prev)
                prev = emit_front(b)
            emit_back(NB - 1, prev)

    nc.compile()
    return nc


def _make_in_maps(inputs: dict) -> list:
    fL = np.asarray(inputs["fL"], dtype=np.float32).astype(ml_dtypes.bfloat16)
    fR = np.asarray(inputs["fR"], dtype=np.float32).astype(ml_dtypes.bfloat16)
    fL = np.ascontiguousarray(fL)
    fR = np.ascontiguousarray(fR)
    return [{"fL": fL[k], "fR": fR[k]} for k in range(N_CORES)]


def kernel(fL: np.ndarray, fR: np.ndarray) -> np.ndarray:
    if "nc" not in _cache:
        _cache["nc"] = _build()
    nc = _cache["nc"]

    in_maps = _make_in_maps({"fL": fL, "fR": fR})
    res = run_bass_kernel_spmd(nc, in_maps, core_ids=list(range(N_CORES)))
    out = np.stack(
        [res.results[k]["out"].astype(np.float32) for k in range(N_CORES)],
        axis=0,
    )
    return out


if __name__ == "__main__":
    rng = np.random.default_rng(0)
    a = rng.standard_normal((N_CORES, C, H, W)).astype(np.float32)
    b = rng.standard_normal((N_CORES, C, H, W)).astype(np.float32)
    o = kernel(a, b)
    print("kernel ran, output shape", o.shape)
